# revision 17
# baseline (speedup 1.0000x reference)
"""Trainium2 Bass kernel for nn_Net_LV (Monte-Carlo local-vol path simulator).

Data-parallel over MC samples: 8 NeuronCores x 4096 samples each.
On-chip: the 90-step recursion (3 fused MLPs per step in a feature-major
layout, heads + pathwise epilogue in a sample-major [128,32] layout).
Host: weight fusion/padding, and all final statistics (means/vars/prices).

Layouts per core (4096 local samples, s = p*32 + c):
  layout A (trunk):   [feature_partition, sample_free]  acts [100, 4096]
  layout B (epilogue):[sample_partition 128, sample_col 32]
Feature rows: diff 0-49, cvv 50-79, cve 80-99.
"""

import numpy as np

N_MAT = 3
PERIOD = 30
NS = N_MAT * PERIOD          # 90
NG = NS + 1                  # 91
NK = 21
RATE = np.float32(0.025)
MC = 32768
NCORES = 8
MCL = MC // NCORES           # 4096
P = 128
C32 = MCL // P               # 32
W = 100                      # fused trunk width
NCH = MCL // 512             # 8  (512-wide layout-A chunks)
NCH128 = MCL // 128          # 32 (128-sample layout-B chunks)

_CACHE = {}


def _mlp_np(Wi, bi, Wh, bh, Wo, bo, x, softplus_out):
    h = np.maximum(x @ Wi + bi, 0.0).astype(np.float32)
    for l in range(Wh.shape[0]):
        h = np.maximum(h @ Wh[l] + bh[l], 0.0).astype(np.float32)
    y = (h @ Wo + bo).astype(np.float32)
    if softplus_out:
        y = (np.log1p(np.exp(-np.abs(y))) + np.maximum(y, 0.0)).astype(np.float32)
    return y


def _build_program():
    import concourse.bass as bass
    import concourse.tile as tile
    from concourse import bacc, mybir

    F32 = mybir.dt.float32
    F32R = mybir.dt.float32r
    AF = mybir.ActivationFunctionType
    ALU = mybir.AluOpType
    from concourse.tile import add_dep_helper

    tg = (np.float32(0.01) * np.arange(NG, dtype=np.float32)).astype(np.float32)

    nc = bacc.Bacc("TRN2", target_bir_lowering=False, debug=False,
                   num_devices=NCORES)

    def din(name, shape, dt=F32):
        return nc.dram_tensor(name, list(shape), dt, kind="ExternalInput").ap()

    def dout(name, shape, dt=F32):
        return nc.dram_tensor(name, list(shape), dt, kind="ExternalOutput").ap()

    z_d = din("zR", [P, NS * C32])
    pinit_d = din("path_init", [NG, MCL], F32R)
    wcve_d = din("wcve", [NG - 0 - 1 + 1, 0 + N_MAT * W])  # [91, 300]
    w1b_d = din("w1b", [1, NS * W])
    w2_d = din("w2f", [W, N_MAT * W])
    w3_d = din("w3f", [W, N_MAT * W])
    w4_d = din("w4f", [50, N_MAT * 50])
    wh_d = din("whf", [W, N_MAT * 64])
    wd_d = din("wdf", [50, N_MAT * 4])
    b1_d = din("b1f", [W, NS])
    b2_d = din("b2f", [W, N_MAT])
    b3_d = din("b3f", [W, N_MAT])
    b4_d = din("b4f", [50, N_MAT])
    bod_d = din("bodf", [P, N_MAT])

    path_o = dout("path_out", [NS, P, C32], F32R)
    vars_o = dout("vars_out", [NS, P, C32])
    cvv_o = dout("cvv_snap", [N_MAT, P, C32, NK])
    g_o = dout("g_snap", [N_MAT, P, C32])
    cve_o = dout("cve_out", [P, C32])

    from contextlib import ExitStack
    with tile.TileContext(nc) as tc, ExitStack() as es:
        sb = es.enter_context(tc.tile_pool(name="sb", bufs=1))
        ep = es.enter_context(tc.tile_pool(name="ep", bufs=2))
        pp1 = es.enter_context(tc.tile_pool(name="pp1", bufs=1, space="PSUM"))
        pp2 = es.enter_context(tc.tile_pool(name="pp2", bufs=1, space="PSUM"))
        pp3 = es.enter_context(tc.tile_pool(name="pp3", bufs=1, space="PSUM"))
        pp4 = es.enter_context(tc.tile_pool(name="pp4", bufs=1, space="PSUM"))
        ppv = es.enter_context(tc.tile_pool(name="ppv", bufs=2, space="PSUM"))
        ppd = es.enter_context(tc.tile_pool(name="ppd", bufs=1, space="PSUM"))

        _li = [0]

        def load_r(shape, src_ap):
            _li[0] += 1
            t = sb.tile(list(shape), F32, name=f"lt{_li[0]}", tag=f"lt{_li[0]}")
            r = sb.tile(list(shape), F32R, name=f"lrr{_li[0]}", tag=f"lrr{_li[0]}")
            nc.sync.dma_start(t[:], src_ap)
            nc.vector.tensor_copy(r[:], t[:])
            return r

        def load_f(shape, src_ap, tag):
            t = sb.tile(list(shape), F32, tag=tag)
            nc.sync.dma_start(t[:], src_ap)
            return t

        wcve_r = load_r((NG, N_MAT * W), wcve_d[:])
        w1b_r = load_r((1, NS * W), w1b_d[:])
        w2_r = load_r((W, N_MAT * W), w2_d[:])
        w3_r = load_r((W, N_MAT * W), w3_d[:])
        w4_r = load_r((50, N_MAT * 50), w4_d[:])
        wh_r = load_r((W, N_MAT * 64), wh_d[:])
        wd_r = load_r((50, N_MAT * 4), wd_d[:])
        b1_s = load_f((W, NS), b1_d[:], "b1")
        b2_s = load_f((W, N_MAT), b2_d[:], "b2")
        b3_s = load_f((W, N_MAT), b3_d[:], "b3")
        b4_s = load_f((50, N_MAT), b4_d[:], "b4")
        bod_s = load_f((P, N_MAT), bod_d[:], "bod")
        z_s = load_f((P, NS * C32), z_d[:], "zs")

        path_A = sb.tile([NG, MCL], F32R, tag="pathA")
        nc.sync.dma_start(path_A[:], pinit_d[:])

        ones_s = sb.tile([P, 1], F32, tag="ones")
        nc.vector.memset(ones_s[:], 1.0)
        cv_s = sb.tile([P, NCH128 * 64], F32, tag="cv")
        nc.vector.memset(cv_s[:], 0.0)
        G_s = sb.tile([P, C32], F32, tag="G")
        nc.vector.memset(G_s[:], 0.0)

        pa1 = es.enter_context(tc.tile_pool(name="pa1", bufs=3))
        pa2 = es.enter_context(tc.tile_pool(name="pa2", bufs=3))
        act3_f = sb.tile([W, MCL], F32R, tag="act3f")
        act4_f = sb.tile([50, MCL], F32R, tag="act4f")

        # S state: ping-pong f32r tiles
        S_tiles = [sb.tile([P, C32], F32, name=f"Stile{k}", tag=f"S{k}")
                   for k in range(2)]

        # initialize S = S0 from path_init row 0 is not directly usable
        # (layout differs); instead DMA from path row 0 rearranged is layout A.
        # Simpler: host also supplies S0 in z-layout via first col of path_o?
        # We initialise S via a dedicated DRAM input:
        s0_d = din("s0_init", [P, C32])
        nc.sync.dma_start(S_tiles[0][:], s0_d[:])
        S_r = sb.tile([P, C32], F32R, tag="Sr")
        s0r_d = din("s0_row", [1, MCL], F32R)
        S_row = sb.tile([1, MCL], F32R, tag="Srow")
        nc.sync.dma_start(S_row[:], s0r_d[:])

        relu_count = [0]

        def relu(dst_ap, src_ap, bias_ap):
            # DVE carries the 32 cv-update STTs per step, so give it only
            # ~9/32 relus; ScalarE takes the rest (balance ~14us/step each).
            idx = relu_count[0] % 32
            relu_count[0] += 1
            if idx % 4 == 1 or idx == 30:
                nc.vector.tensor_scalar(dst_ap, src_ap, bias_ap, 0.0,
                                        op0=ALU.add, op1=ALU.max)
            else:
                nc.scalar.activation(dst_ap, src_ap, AF.Relu,
                                     bias=bias_ap, scale=1.0)

        S_cur = S_tiles[0]
        for i in range(1, NS + 1):
            i0 = i - 1
            m = i0 // PERIOD
            t_prev = tg[i0]
            h = np.float32(tg[i] - tg[i0])
            sqh = np.float32(np.sqrt(h))
            disc = np.float32(np.exp(-RATE * t_prev))
            a1 = float(np.float32(RATE * h))      # RATE*h
            a2 = float(np.float32(RATE * sqh))    # RATE*sqh
            g1 = float(np.float32(disc * sqh))    # disc*sqh
            fsqh = float(sqh)

            m100 = slice(m * W, (m + 1) * W)
            m64 = slice(m * 64, (m + 1) * 64)
            m50 = slice(m * 50, (m + 1) * 50)
            m4 = slice(m * 4, (m + 1) * 4)
            bstep = b1_s[:, i0:i0 + 1]

            # ---- trunk (layout A) ----
            for ch in range(NCH):
                cs = slice(ch * 512, (ch + 1) * 512)
                at1 = pa1.tile([W, 512], F32R, name="at1", tag="a1")
                at2 = pa2.tile([W, 512], F32R, name="at2", tag="a2")
                p1 = pp1.tile([W, 512], F32, tag="p1")
                if i > 1:
                    nc.tensor.matmul(p1[:], wcve_r[0:i - 1, m100],
                                     path_A[0:i - 1, cs],
                                     start=True, stop=False)
                nc.tensor.matmul(p1[:], w1b_r[0:1, i0 * W:(i0 + 1) * W],
                                 S_row[0:1, cs],
                                 start=(i == 1), stop=True)
                relu(at1[:], p1[:], bstep)
                p2 = pp2.tile([W, 512], F32, tag="p2")
                nc.tensor.matmul(p2[:], w2_r[:, m100], at1[:],
                                 start=True, stop=True)
                relu(at2[:], p2[:], b2_s[:, m:m + 1])
                p3 = pp3.tile([W, 512], F32, tag="p3")
                nc.tensor.matmul(p3[:], w3_r[:, m100], at2[:],
                                 start=True, stop=True)
                relu(act3_f[:, cs], p3[:], b3_s[:, m:m + 1])
                p4 = pp4.tile([50, 512], F32, tag="p4")
                nc.tensor.matmul(p4[:], w4_r[:, m50], act3_f[0:50, cs],
                                 start=True, stop=True)
                relu(act4_f[:, cs], p4[:], b4_s[:, m:m + 1])

            # ---- heads ----
            pde = ppd.tile([P, 4 * NCH128], F32, tag="pde")
            for c in range(NCH128):
                nc.tensor.matmul(pde[:, 4 * c:4 * c + 4],
                                 act4_f[:, c::C32],
                                 wd_r[:, m4],
                                 start=(c == 0), stop=(c == NCH128 - 1))
            pvs = []
            for c in range(NCH128):
                pv = ppv.tile([P, 64], F32, tag="pv")
                nc.tensor.matmul(pv[:], act3_f[:, c::C32],
                                 wh_r[:, m64], start=True, stop=True)
                pvs.append(pv)

            # ---- epilogue (layout B [128, 32]) ----
            y_d = pde[:].rearrange("p (c f) -> p c f", f=4)[:, :, 0]
            bodm = bod_s[:, m:m + 1]
            va = ep.tile([P, C32], F32, tag="va")
            nc.scalar.activation(va[:], y_d, AF.Abs, bias=bodm, scale=1.0)
            e_t = ep.tile([P, C32], F32, tag="e")
            nc.scalar.activation(e_t[:], va[:], AF.Exp, scale=-1.0)
            l_t = ep.tile([P, C32], F32, tag="l")
            nc.scalar.activation(l_t[:], e_t[:], AF.Ln, bias=ones_s[:], scale=1.0)
            r_t = ep.tile([P, C32], F32, tag="r")
            nc.scalar.activation(r_t[:], y_d, AF.Relu, bias=bodm, scale=1.0)
            diff = ep.tile([P, C32], F32, tag="diff")
            nc.vector.tensor_add(diff[:], l_t[:], r_t[:])

            vs = ep.tile([P, C32], F32, tag="vs")
            nc.vector.tensor_mul(vs[:], diff[:], diff[:])
            nc.sync.dma_start(vars_o[i0], vs[:])

            Sf = S_cur[:]
            zi = z_s[:, i0 * C32:(i0 + 1) * C32]
            Sd = ep.tile([P, C32], F32, tag="Sd")
            nc.vector.tensor_mul(Sd[:], Sf, diff[:])
            den2 = ep.tile([P, C32], F32, tag="den2")
            nc.vector.tensor_scalar(den2[:], Sd[:], fsqh, 1.0,
                                    op0=ALU.mult, op1=ALU.add)
            r2 = ep.tile([P, C32], F32, tag="r2")
            nc.vector.reciprocal(r2[:], den2[:])
            t1 = ep.tile([P, C32], F32, tag="t1")
            nc.vector.tensor_mul(t1[:], Sd[:], zi)
            Bv = ep.tile([P, C32], F32, tag="Bv")
            nc.vector.scalar_tensor_tensor(Bv[:], t1[:], fsqh, r2[:],
                                           op0=ALU.mult, op1=ALU.mult)
            den1 = ep.tile([P, C32], F32, tag="den1")
            nc.vector.tensor_scalar(den1[:], Sf, a2, 1.0,
                                    op0=ALU.mult, op1=ALU.add)
            r1 = ep.tile([P, C32], F32, tag="r1")
            nc.vector.reciprocal(r1[:], den1[:])
            Av = ep.tile([P, C32], F32, tag="Av")
            nc.vector.scalar_tensor_tensor(Av[:], Sf, a1, r1[:],
                                           op0=ALU.mult, op1=ALU.mult)
            t2 = ep.tile([P, C32], F32, tag="t2")
            nc.vector.tensor_add(t2[:], Av[:], Bv[:])
            S_new = S_tiles[i % 2]
            nc.vector.tensor_add(S_new[:], t2[:], Sf)

            nc.vector.tensor_copy(S_r[:], S_new[:])
            d1 = nc.sync.dma_start(path_o[i0], S_r[:])
            if i < NS:
                rowsrc = path_o.tensor.ap()[i0:i0 + 1].rearrange(
                    "i p c -> i (p c)")
                d2 = nc.sync.dma_start(path_A[i:i + 1, :], rowsrc)
                d3 = nc.sync.dma_start(S_row[:], rowsrc)
                _ = (d1, d2, d3)

            g_t = ep.tile([P, C32], F32, tag="g")
            nc.vector.tensor_scalar(g_t[:], t1[:], g1, None, op0=ALU.mult)
            nc.vector.tensor_add(G_s[:], G_s[:], g_t[:])

            for c in range(NCH128):
                cvsl = cv_s[:, c * 64:(c + 1) * 64]
                nc.vector.scalar_tensor_tensor(cvsl, pvs[c][:],
                                               g_t[:, c:c + 1], cvsl,
                                               op0=ALU.mult, op1=ALU.add)

            S_cur = S_new

            if i % PERIOD == 0:
                cv3 = cv_s[:].rearrange("p (c k) -> p c k", k=64)
                nc.sync.dma_start(cvv_o[m], cv3[:, :, m * NK:(m + 1) * NK])
                nc.sync.dma_start(g_o[m], G_s[:])

        cv3 = cv_s[:].rearrange("p (c k) -> p c k", k=64)
        nc.sync.dma_start(cve_o[:], cv3[:, :, 63])

    nc.compile()
    return nc


def _prep_inputs(inputs):
    """Host-side: fused/padded weight tensors + per-core z shards."""
    f = np.float32
    S0 = f(np.asarray(inputs["S0"]).reshape(-1)[0])
    z = np.asarray(inputs["z"], dtype=np.float32)

    dWi = np.asarray(inputs["diff_Wi"], np.float32)
    dbi = np.asarray(inputs["diff_bi"], np.float32)
    dWh = np.asarray(inputs["diff_Wh"], np.float32)
    dbh = np.asarray(inputs["diff_bh"], np.float32)
    dWo = np.asarray(inputs["diff_Wo"], np.float32)
    dbo = np.asarray(inputs["diff_bo"], np.float32)
    vWi = np.asarray(inputs["cvv_Wi"], np.float32)
    vbi = np.asarray(inputs["cvv_bi"], np.float32)
    vWh = np.asarray(inputs["cvv_Wh"], np.float32)
    vbh = np.asarray(inputs["cvv_bh"], np.float32)
    vWo = np.asarray(inputs["cvv_Wo"], np.float32)
    vbo = np.asarray(inputs["cvv_bo"], np.float32)
    eWi = np.asarray(inputs["cve_Wi"], np.float32)
    ebi = np.asarray(inputs["cve_bi"], np.float32)
    eWh = np.asarray(inputs["cve_Wh"], np.float32)
    ebh = np.asarray(inputs["cve_bh"], np.float32)
    eWo = np.asarray(inputs["cve_Wo"], np.float32)
    ebo = np.asarray(inputs["cve_bo"], np.float32)

    tg = (f(0.01) * np.arange(NG, dtype=np.float32)).astype(np.float32)

    # rows: diff 0-49, cvv 50-79, cve 80-99
    wcve = np.zeros((NG, N_MAT * W), np.float32)
    w1b = np.zeros((1, NS * W), np.float32)
    w2f = np.zeros((W, N_MAT * W), np.float32)
    w3f = np.zeros((W, N_MAT * W), np.float32)
    w4f = np.zeros((50, N_MAT * 50), np.float32)
    whf = np.zeros((W, N_MAT * 64), np.float32)
    wdf = np.zeros((50, N_MAT * 4), np.float32)
    b1f = np.zeros((W, NS), np.float32)
    b2f = np.zeros((W, N_MAT), np.float32)
    b3f = np.zeros((W, N_MAT), np.float32)
    b4f = np.zeros((50, N_MAT), np.float32)
    bodf = np.zeros((P, N_MAT), np.float32)

    for m in range(N_MAT):
        wcve[0:NG - 0, m * W + 80:m * W + 100][:, :] = 0.0
        wcve[0:91, m * W + 80:m * W + 100] = eWi[m][1:92, :]
        w2f[0:50, m * W + 0:m * W + 50] = dWh[m, 0]
        w2f[50:80, m * W + 50:m * W + 80] = vWh[m, 0]
        w2f[80:100, m * W + 80:m * W + 100] = eWh[m, 0]
        w3f[0:50, m * W + 0:m * W + 50] = dWh[m, 1]
        w3f[50:80, m * W + 50:m * W + 80] = vWh[m, 1]
        w3f[80:100, m * W + 80:m * W + 100] = eWh[m, 1]
        w4f[:, m * 50:(m + 1) * 50] = dWh[m, 2]
        whf[50:80, m * 64:m * 64 + 63] = vWo[m]
        whf[80:100, m * 64 + 63:m * 64 + 64] = eWo[m]
        wdf[:, m * 4:m * 4 + 1] = dWo[m]
        b2f[0:50, m] = dbh[m, 0]
        b2f[50:80, m] = vbh[m, 0]
        b2f[80:100, m] = ebh[m, 0]
        b3f[0:50, m] = dbh[m, 1]
        b3f[50:80, m] = vbh[m, 1]
        b3f[80:100, m] = ebh[m, 1]
        b4f[:, m] = dbh[m, 2]
        bodf[:, m] = dbo[m, 0]

    for i in range(1, NS + 1):
        i0 = i - 1
        m = i0 // PERIOD
        t = tg[i0]
        w1b[0, i0 * W + 0:i0 * W + 50] = dWi[m][1, :]
        w1b[0, i0 * W + 50:i0 * W + 80] = vWi[m][1, :]
        w1b[0, i0 * W + 80:i0 * W + 100] = eWi[m][i, :]
        b1f[0:50, i0] = (dbi[m] + t * dWi[m][0, :]).astype(np.float32)
        b1f[50:80, i0] = (vbi[m] + t * vWi[m][0, :]).astype(np.float32)
        b1f[80:100, i0] = (ebi[m] + t * eWi[m][0, :]).astype(np.float32)

    path_init = np.zeros((NG, MCL), np.float32)
    path_init[0, :] = S0
    s0_init = np.full((P, C32), S0, np.float32)

    s0_row = np.full((1, MCL), S0, np.float32)
    shared = dict(wcve=wcve, w1b=w1b, w2f=w2f, w3f=w3f, w4f=w4f, whf=whf,
                  wdf=wdf, b1f=b1f, b2f=b2f, b3f=b3f, b4f=b4f, bodf=bodf,
                  path_init=path_init, s0_init=s0_init, s0_row=s0_row)

    in_maps = []
    for k in range(NCORES):
        zc = z[k * MCL:(k + 1) * MCL]                 # [4096, 90]
        zR = (zc.reshape(P, C32, NS).transpose(0, 2, 1)
                .reshape(P, NS * C32).copy())         # [128, 90*32]
        m = dict(shared)
        m["zR"] = np.ascontiguousarray(zR)
        in_maps.append(m)

    host = dict(S0=S0, tg=tg, dWi=dWi, dbi=dbi, dWh=dWh, dbh=dbh, dWo=dWo,
                dbo=dbo, vbo=vbo, ebo=ebo)
    return in_maps, host


def kernel(**inputs):
    from concourse import bass_utils

    if "prog" not in _CACHE:
        _CACHE["prog"] = _build_program()
    nc = _CACHE["prog"]

    in_maps, host = _prep_inputs(inputs)
    res = bass_utils.run_bass_kernel_spmd(nc, in_maps,
                                          core_ids=list(range(NCORES)))
    f = np.float32
    S0, tg = host["S0"], host["tg"]
    dWi, dbi, dWh, dbh, dWo, dbo = (host[k] for k in
                                    ("dWi", "dbi", "dWh", "dbh", "dWo", "dbo"))
    vbo, ebo = host["vbo"], host["ebo"]

    path = np.zeros((MC, NG), np.float32)
    var_path = np.zeros((MC, NG), np.float32)
    cv_snap = np.zeros((N_MAT, MC, NK), np.float32)
    G_snap = np.zeros((N_MAT, MC), np.float32)
    cv_e = np.zeros((MC, 1), np.float32)

    # var0 / path0 (identical across samples)
    x0 = np.array([[0.0, S0]], np.float32)
    d0 = _mlp_np(dWi[0], dbi[0], dWh[0], dbh[0], dWo[0], dbo[0], x0, True)
    var0 = f(d0[0, 0]) ** 2

    path[:, 0] = S0
    var_path[:, 0] = var0
    for k in range(NCORES):
        o = res.results[k]
        sl = slice(k * MCL, (k + 1) * MCL)
        # [90, 128, 32] -> [4096, 90] with s = p*32+c
        po = np.asarray(o["path_out"], np.float32).reshape(NS, MCL).T
        vo = np.asarray(o["vars_out"], np.float32).reshape(NS, MCL).T
        path[sl, 1:] = po
        var_path[sl, 1:] = vo
        cv_snap[:, sl, :] = np.asarray(o["cvv_snap"], np.float32).reshape(
            N_MAT, MCL, NK)
        G_snap[:, sl] = np.asarray(o["g_snap"], np.float32).reshape(N_MAT, MCL)
        cv_e[sl, 0] = np.asarray(o["cve_out"], np.float32).reshape(MCL)

    # bias corrections: cv += sum_m' dG_m' * bo[m']
    dG = np.empty_like(G_snap)
    dG[0] = G_snap[0]
    dG[1] = G_snap[1] - G_snap[0]
    dG[2] = G_snap[2] - G_snap[1]
    strikes = (f(0.8) + f(0.02) * np.arange(NK, dtype=np.float32)).astype(f)

    price_v = np.zeros((N_MAT, NK), np.float32)
    varp_v = np.zeros((N_MAT, NK), np.float32)
    for m in range(N_MAT):
        cv_true = cv_snap[m].copy()
        for mp in range(m + 1):
            # cvv_bo[mp] columns for strike-block m
            cv_true += dG[mp][:, None] * host_vbo_block(vbo, mp, m)
        S_T = path[:, (m + 1) * PERIOD]
        tm = tg[(m + 1) * PERIOD]
        disc = f(np.exp(-RATE * tm))
        pr = disc * np.maximum(S_T[:, None] - strikes[None, :], 0.0) - cv_true
        pr = pr.astype(np.float32)
        price_v[m] = pr.mean(0, dtype=np.float32)
        varp_v[m] = pr.var(0, ddof=1, dtype=np.float32)

    # cv_e bias: + sum_m dG_m * ebo[m]
    cv_e = cv_e + (dG * ebo.reshape(N_MAT, 1)[:, 0][:, None]).sum(0)[:, None]
    cv_e = cv_e.astype(np.float32)

    rmax = path.max(axis=1)
    S_last = path[:, NS]
    exotic = (rmax - S_last)[:, None].astype(np.float32)
    discT = f(np.exp(-RATE * tg[NS]))
    exotic_price = (discT * exotic - cv_e).astype(np.float32)
    de = (discT * exotic).astype(np.float32)
    error = (de - de.mean(dtype=np.float32) - cv_e).astype(np.float32)
    diffusion_last = np.sqrt(var_path[:, NS])[:, None].astype(np.float32)

    return (path, var_path, diffusion_last, price_v, varp_v, exotic_price,
            np.float32(exotic_price.mean(dtype=np.float32)),
            np.float32(exotic_price.var(ddof=1, dtype=np.float32)),
            error)


def host_vbo_block(vbo, mp, m):
    # cvv output column block for maturity m under period-mp weights
    return vbo[mp][m * NK:(m + 1) * NK][None, :]
